# revision 23
# baseline (speedup 1.0000x reference)
"""Single-head attention (B=8, S=2048, E=1024, H=128) with softmax + deterministic
dropout, data-parallel over batch across 8 NeuronCores (one batch element per core).

Per-core layout ("transposed attention"):
  - host ships x twice, pre-arranged to [128, NE, S] so every DMA row is >=2KB
    contiguous: x8 fp8e4m3 (q,k projections) and xT fp16 (v projection).
    Weights are pre-arranged to [128, NE, H]; wq/wk are pre-scaled by 32 so fp8
    values avoid the subnormal range (the 32*32 factor is folded into the exp
    scale) and cast to fp8; wv stays fp16.
  - q,k projections are fp8 DoubleRow matmuls (K=256/instruction, 2x PE rate).
  - v is computed directly in natural [t, h] layout, t-block-major (8 K=128
    N=128 matmuls per t-block into a double-buffered PSUM bank) — no PE
    transposes, only 2 PSUM banks, and t-blocks become available in the order
    the AV matmuls consume them.
  - attention per s-group (512 query columns) in t-pairs: one 2-bank PSUM tile
    [128, 2, 512] takes two QK matmuls; one wide ACT exp -> expT fp16; one wide
    DVE multiply applies the fp8 {0,1} dropout mask; AV matmuls accumulate
    out^T [h, s] in fp32 PSUM.
  - denominator: M=1 ones-matmuls per expT chunk, in waves of 4 packed into
    distinct PE column groups, emitted 2+ pairs late so all operands are ready
    and the 4 matmuls issue back-to-back (they then overlap in the PE array).
  - normalize: den partials sit in PSUM rows {0,32,64,96}; one select matmul
    (sel values = 0.9, folding in the dropout 1/(1-p)) sums them and
    broadcasts 0.9*den[s] to all partitions; reciprocal_approx_fast + one DVE
    multiply rescale out^T during PSUM eviction.  Output stays [H, S] fp16;
    the host transposes back to [S, H] fp32.
  - s-group 0 is emitted specially: all 8 QK pairs first, then the v
    projection, then den waves and AV pairs — so the PE never head-of-line
    blocks on v's xT DMA dependency while q,k attention work is available.

DMA: split across the two TRN2 HWDGE queues (Scalar engine: weights + x8;
Sync: mask + xT + outputs), with the first mask fetch split in halves so
s-group 0 can start as early as possible.
"""

import sys

for _p in ("/opt/trn_rl_repo",):
    if _p not in sys.path:
        sys.path.append(_p)

import numpy as np
import ml_dtypes

B, S, E, H = 8, 2048, 1024, 128
DROP_P = 0.1
P = 128
W_SCALE = 32.0  # host premultiplies wq/wk by this before fp8 cast

_program_cache = {}


def _build_program(S=S, E=E):
    key = (S, E)
    if key in _program_cache:
        return _program_cache[key]
    NT = S // P       # t-chunks (16)
    NE = E // P       # e-chunks (8)
    NEP = NE // 2     # e-pairs for DoubleRow (4)
    SG = 512          # s-group width (one fp32 PSUM bank)
    NSG = S // SG     # 4
    NPAIR = NT // 2   # t-pairs per s-group (8)

    import concourse.bass as bass  # noqa: F401
    import concourse.mybir as mybir
    import concourse.tile as tile
    from concourse import bacc
    from concourse.masks import make_identity

    f32 = mybir.dt.float32
    f16 = mybir.dt.float16
    f8 = mybir.dt.float8e4
    Exp = mybir.ActivationFunctionType.Exp
    DR = mybir.MatmulPerfMode.DoubleRow
    SCALE = float(E) ** -0.5 / (W_SCALE * W_SCALE)

    nc = bacc.Bacc("TRN2", target_bir_lowering=False, debug=False)
    # all inputs pre-arranged host-side for contiguous per-partition DMA rows
    x8_d = nc.dram_tensor("x8", [P, NE, S], f8, kind="ExternalInput").ap()
    xT_d = nc.dram_tensor("xT", [P, NE, S], f16, kind="ExternalInput").ap()
    keep_d = nc.dram_tensor("keep8", [P, NT, S], f8, kind="ExternalInput").ap()
    wq8_d = nc.dram_tensor("wq8", [P, NE, H], f8, kind="ExternalInput").ap()
    wk8_d = nc.dram_tensor("wk8", [P, NE, H], f8, kind="ExternalInput").ap()
    wv_d = nc.dram_tensor("wv", [P, NE, H], f16, kind="ExternalInput").ap()
    outT_d = nc.dram_tensor("outT", [H, S], f16, kind="ExternalOutput").ap()

    with tile.TileContext(nc) as tc:
        with (
            tc.tile_pool(name="consts", bufs=1) as consts,
            tc.tile_pool(name="xw", bufs=1) as xw_pool,
        ):
            identity16 = consts.tile([P, P], f16)
            make_identity(nc, identity16)
            ones_t = consts.tile([P, 1], f16)
            nc.vector.memset(ones_t, 1.0)
            # sel128[k, m] = 0.9 for k in {0,32,64,96}: one matmul sums the 4
            # denominator partial rows, folds in the dropout 1/(1-p) factor,
            # and broadcasts to all partitions.
            sel128 = consts.tile([P, P], f16)
            nc.vector.memset(sel128, 0.0)
            for j in range(4):
                nc.vector.memset(sel128[32 * j:32 * j + 1, :], 1.0 - DROP_P)

            # -------- input DMAs --------
            w8_js = []
            for j in range(2):
                wj = xw_pool.tile([P, NE, H], f8, tag=f"w8{j}", name=f"w8{j}")
                w8_js.append(wj)
            wv_sb = xw_pool.tile([P, NE, H], f16, tag="wv", name="wv")
            x8_sb = xw_pool.tile([P, NE, S], f8, tag="x8", name="x8")
            xT_sb = xw_pool.tile([P, NE, S], f16, tag="xT", name="xT")
            qkT_sb = xw_pool.tile([P, 2, S], f16, tag="qkT", name="qkT")
            v_sb = xw_pool.tile([P, NT, H], f16, tag="v", name="v")

            # qSP queue starts issuing ~2us before qAct, so the matmul-gating
            # loads (w8, x8) go there; qAct takes wv + the tail of xT.
            nc.sync.dma_start(w8_js[0], wq8_d)
            nc.sync.dma_start(w8_js[1], wk8_d)
            for ep in range(NEP):
                nc.sync.dma_start(
                    x8_sb[:, 2 * ep:2 * ep + 2, :], x8_d[:, 2 * ep:2 * ep + 2, :]
                )
            nc.scalar.dma_start(wv_sb, wv_d)
            for ep in range(2, NEP):
                nc.scalar.dma_start(
                    xT_sb[:, 2 * ep:2 * ep + 2, :], xT_d[:, 2 * ep:2 * ep + 2, :]
                )

            keeps = {}

            def fetch_keep(sg, pool, engine, half=None):
                if sg not in keeps:
                    keeps[sg] = pool.tile([P, NT, SG], f8, tag="keep",
                                          name=f"keep{sg}")
                t_ = keeps[sg]
                sl = slice(sg * SG, (sg + 1) * SG)
                if half is None:
                    engine.dma_start(t_, keep_d[:, :, sl])
                elif half == 0:
                    engine.dma_start(t_[:, 0:NT // 2, :],
                                     keep_d[:, 0:NT // 2, sl])
                else:
                    engine.dma_start(t_[:, NT // 2:NT, :],
                                     keep_d[:, NT // 2:NT, sl])

            with tc.tile_pool(name="keep_pool", bufs=3) as keep_pool:
                # first half of xT on qSP (behind x8), then the fp8 mask
                # fetches (small enough to land before they're consumed)
                for ep in range(0, 2):
                    nc.sync.dma_start(
                        xT_sb[:, 2 * ep:2 * ep + 2, :], xT_d[:, 2 * ep:2 * ep + 2, :]
                    )
                fetch_keep(0, keep_pool, nc.sync, half=0)
                fetch_keep(0, keep_pool, nc.sync, half=1)
                fetch_keep(1, keep_pool, nc.sync)
                fetch_keep(2, keep_pool, nc.sync)
                fetch_keep(3, keep_pool, nc.sync)

                # -------- q,k projections: fp8 DoubleRow, e-pair-major -----
                with tc.tile_pool(name="proj_ps", bufs=8, space="PSUM") as proj_ps:
                    ps_qk = [
                        proj_ps.tile([P, SG], f32, tag=f"pqk{j}{c}",
                                     name=f"pqk{j}{c}", bufs=1)
                        for j in range(2) for c in range(NSG)
                    ]
                    for ep in range(NEP):
                        for j in range(2):
                            for c in range(NSG):
                                nc.tensor.matmul(
                                    ps_qk[j * NSG + c],
                                    w8_js[j][:, 2 * ep:2 * ep + 2, :],
                                    x8_sb[:, 2 * ep:2 * ep + 2,
                                          c * SG:(c + 1) * SG],
                                    start=(ep == 0),
                                    stop=(ep == NEP - 1),
                                    perf_mode=DR,
                                )
                    for j in range(2):
                        for c in range(NSG):
                            nc.any.tensor_copy(
                                qkT_sb[:, j, c * SG:(c + 1) * SG],
                                ps_qk[j * NSG + c],
                            )

                # -------- v projection: fp16 e-major (xT-pair-paced) --------
                # e iterated in DMA-arrival order (qAct pairs 2,3 land first)
                vT_sb = xw_pool.tile([P, S], f16, tag="vT", name="vT")
                V_E_ORDER = [4, 5, 6, 7, 0, 1, 2, 3]
                with tc.tile_pool(name="vproj_ps", bufs=2, space="PSUM") as vproj_ps:
                    ps_vs = [vproj_ps.tile([P, SG], f32, tag=f"pv{c}",
                                           name=f"pv{c}", bufs=1)
                             for c in range(NSG)]
                    for ei, e in enumerate(V_E_ORDER):
                        for c in range(NSG):
                            nc.tensor.matmul(
                                ps_vs[c],
                                wv_sb[:, e, :],
                                xT_sb[:, e, c * SG:(c + 1) * SG],
                                start=(ei == 0),
                                stop=(ei == NE - 1),
                            )
                    # evict + transpose per column group so v_sb chunks become
                    # available incrementally
                    for g in range(NSG):
                        nc.any.tensor_copy(vT_sb[:, g * SG:(g + 1) * SG], ps_vs[g])
                        ps_t = vproj_ps.tile([P, 4, P], f16, tag="ptr", name="ptr")
                        for j in range(4):
                            nc.tensor.transpose(
                                ps_t[:, j, :],
                                vT_sb[:, (4 * g + j) * P:(4 * g + j + 1) * P],
                                identity16,
                            )
                        nc.any.tensor_copy(v_sb[:, 4 * g:4 * g + 4, :], ps_t)

                # -------- attention loop over s-groups --------
                with (
                    tc.tile_pool(name="att_ps", bufs=3, space="PSUM") as att_ps,
                    tc.tile_pool(name="acc_ps", bufs=1, space="PSUM") as acc_ps,
                    tc.tile_pool(name="work", bufs=2) as work_pool,
                ):
                    for sg in range(NSG):
                        s_sl = slice(sg * SG, (sg + 1) * SG)
                        keep_sg = keeps.pop(sg)
                        psum_out = acc_ps.tile([P, SG], f32, tag="out")
                        psum_den = acc_ps.tile([P, SG], f32, tag="den")
                        # zero rows the den waves don't write, so the select
                        # matmul only sees finite values
                        nc.vector.memset(psum_den, 0.0)
                        expTs = {}
                        attds = {}

                        def emit_pair(i, s_sl=s_sl, keep_sg=keep_sg,
                                      expTs=expTs, attds=attds):
                            ps = att_ps.tile([P, 2, SG], f32, tag="att",
                                             name=f"att{i}")
                            for h_ in range(2):
                                t = 2 * i + h_
                                nc.tensor.matmul(
                                    ps[:, h_, :],
                                    qkT_sb[:, 1, t * P:(t + 1) * P],
                                    qkT_sb[:, 0, s_sl],
                                    start=True,
                                    stop=True,
                                )
                            expT = work_pool.tile([P, 2, SG], f16, tag="exp",
                                                  name=f"exp{i}", bufs=7)
                            nc.scalar.activation(expT, ps, Exp, scale=SCALE)
                            attd = work_pool.tile([P, 2, SG], f16, tag="attd",
                                                  name=f"attd{i}", bufs=3)
                            # alternate mask-multiply between DVE and the
                            # otherwise-idle GpSimd so neither paces the loop
                            eng = nc.vector if i % 2 == 0 else nc.gpsimd
                            eng.tensor_mul(
                                out=attd, in0=expT,
                                in1=keep_sg[:, 2 * i:2 * i + 2, :],
                            )
                            expTs[i] = expT
                            attds[i] = attd

                        def emit_av_pair(i, psum_out=psum_out, attds=attds):
                            attd = attds.pop(i)
                            for h_ in range(2):
                                t = 2 * i + h_
                                nc.tensor.matmul(
                                    psum_out,
                                    v_sb[:, t, :],
                                    attd[:, h_, :],
                                    start=(t == 0),
                                    stop=(t == NT - 1),
                                )

                        def emit_den_wave(w, psum_den=psum_den, expTs=expTs):
                            e0 = expTs.pop(2 * w)
                            e1 = expTs.pop(2 * w + 1)
                            for j in range(4):
                                src = (e0 if j < 2 else e1)[:, j % 2, :]
                                nc.tensor.matmul(
                                    psum_den[32 * j:32 * j + 1, :],
                                    ones_t,
                                    src,
                                    start=(w == 0),
                                    stop=(w == NPAIR // 2 - 1),
                                    tile_position=(0, 32 * j),
                                )

                        # software pipeline: av lags 1 pair, den waves lag so
                        # all 4 packed matmuls are ready when issued
                        for i in range(NPAIR):
                            emit_pair(i)
                            if i >= 1:
                                emit_av_pair(i - 1)
                            if i >= 5 and i % 2 == 1:
                                emit_den_wave((i - 5) // 2)
                        emit_av_pair(NPAIR - 1)
                        emit_den_wave(NPAIR // 2 - 2)
                        emit_den_wave(NPAIR // 2 - 1)

                        # ---- normalize + output (transposed layout) ----
                        den_all = work_pool.tile([P, SG], f16, tag="den_all")
                        nc.vector.tensor_copy(den_all, psum_den)
                        nc.tensor.matmul(
                            psum_den, sel128, den_all, start=True, stop=True)
                        recip_sb = work_pool.tile([P, SG], f32, tag="recip")
                        nc.vector.reciprocal_approx_fast(
                            out=recip_sb, in_=psum_den)
                        out_sb = work_pool.tile([P, SG], f16, tag="out_sb")
                        nc.vector.tensor_mul(
                            out=out_sb, in0=psum_out, in1=recip_sb)
                        nc.sync.dma_start(outT_d[:, s_sl], out_sb)

    nc.compile()
    _program_cache[key] = nc
    return nc


def kernel(x, wq, wk, wv, drop_u):
    from concourse import bass_utils

    x = np.asarray(x)
    wq = np.asarray(wq)
    wk = np.asarray(wk)
    wv = np.asarray(wv)
    drop_u = np.asarray(drop_u)

    nc = _build_program()
    in_maps = build_in_maps(x, wq, wk, wv, drop_u)
    last_err = None
    for _attempt in range(3):
        try:
            res = bass_utils.run_bass_kernel_spmd(
                nc, in_maps, core_ids=list(range(B)), trace=False
            )
            return np.stack(
                [np.asarray(res.results[b]["outT"]).T.astype(np.float32)
                 for b in range(B)],
                axis=0,
            )
        except Exception as e:  # transient device errors — retry
            last_err = e
            import time as _time

            _time.sleep(2.0)
    raise last_err


def _arrange_pe(a, ne):
    """[E, N] -> [128, ne, N] with e-chunk rows contiguous per partition."""
    E_, N_ = a.shape
    return np.ascontiguousarray(a.reshape(ne, P, N_).transpose(1, 0, 2))


def build_in_maps(x, wq, wk, wv, drop_u):
    f8 = ml_dtypes.float8_e4m3
    NE = E // P
    NT = S // P
    wq8 = _arrange_pe((np.asarray(wq) * W_SCALE).astype(f8), NE)
    wk8 = _arrange_pe((np.asarray(wk) * W_SCALE).astype(f8), NE)
    wv16 = _arrange_pe(np.asarray(wv).astype(np.float16), NE)
    in_maps = []
    for b in range(B):
        xTb = np.ascontiguousarray(x[b].T)
        x8 = _arrange_pe(xTb.astype(f8), NE)
        xT = _arrange_pe(xTb.astype(np.float16), NE)
        keep8 = _arrange_pe(
            (drop_u[b].T >= np.float32(DROP_P)).astype(f8), NT)
        in_maps.append(
            {"x8": x8, "xT": xT, "keep8": keep8,
             "wq8": wq8, "wk8": wk8, "wv": wv16}
        )
    return in_maps


# revision 24
# speedup vs baseline: 1.0792x; 1.0792x over previous
"""Single-head attention (B=8, S=2048, E=1024, H=128) with softmax + deterministic
dropout, data-parallel over batch across 8 NeuronCores (one batch element per core).

Layout ("transposed attention"), engineered so the ACT engine (which does the
4M-element exp() per core, the true throughput floor at ~27us) never bubbles:

  - host ships x twice, pre-arranged [128, NE, S]: x8 fp8e4m3 (q,k projections,
    DoubleRow K=256 matmuls at 2x PE rate) and xT fp16 (v projection).  wq/wk
    are pre-scaled by 32 (fp8 subnormal avoidance; folded into the exp scale)
    and shipped fp8; wv fp16.  All weight tensors pre-arranged [128, NE, H].
  - dropout mask shipped as {0,1} fp16 [128, NT, S]: exact, and fp16 keeps the
    wide DVE multiply in 2x 16-bit mode (~683ns/pair; fp8 masks drop it to 1x).
    The 1/(1-p) factor is folded into the denominator select matmul (0.9).
  - attention runs as one flat stream of t-pairs across all 4 s-groups: per
    pair one 2-bank PSUM tile takes two QK matmuls, one wide ACT exp makes
    expT fp16, one wide DVE multiply applies the mask.  The per-s-group
    post-work (denominator waves, AV matmuls, normalize) is emitted AFTER the
    NEXT s-group's QK pairs, so the PE always has pair-matmuls ready and the
    ACT exp pipeline stays saturated across s-group boundaries.
  - denominator: M=1 ones-matmuls per expT chunk in waves of 4 packed into
    distinct PE column groups (tile_position), emitted when all operands are
    long-ready so they issue back-to-back and overlap in the array.
  - v projection: e-major fp16 matmuls into the out/den PSUM banks (which are
    not yet needed), emitted after s-group 0's QK pairs so it never blocks
    them; PE transposes (scratch = bitcast of the same banks) produce natural
    v [t, h] for the AV matmuls.
  - normalize: den partials in PSUM rows {0,32,64,96}; one select matmul sums
    and broadcasts 0.9*den[s]; reciprocal_approx_fast (single custom DVE op)
    + one DVE multiply rescale out^T during eviction.  Output stays [H, S]
    fp16; host transposes to [S, H] fp32.

PSUM budget (8 banks): 3 x [128,2,512] QK tiles (6) + out (1) + den (1); the
v projection and transpose scratch time-share out/den before the AV matmuls.

DMA: critical stream (w8, x8, xT tail) on the Scalar-engine HWDGE queue;
mask + xT head + outputs on the Sync queue, ordered by first-use time.
"""

import sys

for _p in ("/opt/trn_rl_repo",):
    if _p not in sys.path:
        sys.path.append(_p)

import numpy as np
import ml_dtypes

B, S, E, H = 8, 2048, 1024, 128
DROP_P = 0.1
P = 128
W_SCALE = 32.0  # host premultiplies wq/wk by this before fp8 cast

_program_cache = {}


def _build_program(S=S, E=E):
    key = (S, E)
    if key in _program_cache:
        return _program_cache[key]
    NT = S // P       # t-chunks (16)
    NE = E // P       # e-chunks (8)
    NEP = NE // 2     # e-pairs for DoubleRow (4)
    SG = 512          # s-group width (one fp32 PSUM bank)
    NSG = S // SG     # 4
    NPAIR = NT // 2   # t-pairs per s-group (8)

    import concourse.bass as bass  # noqa: F401
    import concourse.mybir as mybir
    import concourse.tile as tile
    from concourse import bacc
    from concourse.masks import make_identity

    f32 = mybir.dt.float32
    f16 = mybir.dt.float16
    f8 = mybir.dt.float8e4
    Exp = mybir.ActivationFunctionType.Exp
    DR = mybir.MatmulPerfMode.DoubleRow
    SCALE = float(E) ** -0.5 / (W_SCALE * W_SCALE)

    nc = bacc.Bacc("TRN2", target_bir_lowering=False, debug=False)
    x8_d = nc.dram_tensor("x8", [P, NE, S], f8, kind="ExternalInput").ap()
    xT_d = nc.dram_tensor("xT", [P, NE, S], f16, kind="ExternalInput").ap()
    keep_d = nc.dram_tensor("keep", [P, NT, S], f16, kind="ExternalInput").ap()
    wq8_d = nc.dram_tensor("wq8", [P, NE, H], f8, kind="ExternalInput").ap()
    wk8_d = nc.dram_tensor("wk8", [P, NE, H], f8, kind="ExternalInput").ap()
    wv_d = nc.dram_tensor("wv", [P, NE, H], f16, kind="ExternalInput").ap()
    outT_d = nc.dram_tensor("outT", [H, S], f16, kind="ExternalOutput").ap()

    with tile.TileContext(nc) as tc:
        with (
            tc.tile_pool(name="consts", bufs=1) as consts,
            tc.tile_pool(name="xw", bufs=1) as xw_pool,
            tc.tile_pool(name="keep_pool", bufs=2) as keep_pool,
        ):
            identity16 = consts.tile([P, P], f16)
            make_identity(nc, identity16)
            ones_t = consts.tile([P, 1], f16)
            nc.vector.memset(ones_t, 1.0)
            sel128 = consts.tile([P, P], f16)
            nc.vector.memset(sel128, 0.0)
            for j in range(4):
                nc.vector.memset(sel128[32 * j:32 * j + 1, :], 1.0 - DROP_P)

            w8_js = []
            for j in range(2):
                wj = xw_pool.tile([P, NE, H], f8, tag=f"w8{j}", name=f"w8{j}")
                w8_js.append(wj)
            wv_sb = xw_pool.tile([P, NE, H], f16, tag="wv", name="wv")
            x8_sb = xw_pool.tile([P, NE, S], f8, tag="x8", name="x8")
            xT_sb = xw_pool.tile([P, NE, S], f16, tag="xT", name="xT")
            qkT_sb = xw_pool.tile([P, 2, S], f16, tag="qkT", name="qkT")
            vT_sb = xw_pool.tile([P, S], f16, tag="vT", name="vT")
            v_sb = xw_pool.tile([P, NT, H], f16, tag="v", name="v")

            # qAct (scalar) queue: the matmul-gating stream + xT tail
            nc.scalar.dma_start(w8_js[0], wq8_d)
            nc.scalar.dma_start(w8_js[1], wk8_d)
            for ep in range(NEP):
                nc.scalar.dma_start(
                    x8_sb[:, 2 * ep:2 * ep + 2, :], x8_d[:, 2 * ep:2 * ep + 2, :]
                )
            for ep in (2, 3):
                nc.scalar.dma_start(
                    xT_sb[:, 2 * ep:2 * ep + 2, :], xT_d[:, 2 * ep:2 * ep + 2, :]
                )

            keeps = {}

            def fetch_keep(sg, engine, half=None):
                if sg not in keeps:
                    keeps[sg] = keep_pool.tile([P, NT, SG], f16, tag="keep",
                                               name=f"keep{sg}")
                t_ = keeps[sg]
                sl = slice(sg * SG, (sg + 1) * SG)
                if half is None:
                    engine.dma_start(t_, keep_d[:, :, sl])
                else:
                    rows = slice(0, NT // 2) if half == 0 else slice(NT // 2, NT)
                    engine.dma_start(t_[:, rows, :], keep_d[:, rows, sl])

            # qSP (sync) queue, in first-use order
            nc.sync.dma_start(wv_sb, wv_d)
            fetch_keep(0, nc.sync, half=0)
            fetch_keep(0, nc.sync, half=1)
            for ep in (0, 1):
                nc.sync.dma_start(
                    xT_sb[:, 2 * ep:2 * ep + 2, :], xT_d[:, 2 * ep:2 * ep + 2, :]
                )
            fetch_keep(1, nc.sync)
            fetch_keep(2, nc.sync)
            fetch_keep(3, nc.sync)

            # -------- q,k projections: fp8 DoubleRow, e-pair-major --------
            with tc.tile_pool(name="proj_ps", bufs=8, space="PSUM") as proj_ps:
                ps_qk = [
                    proj_ps.tile([P, SG], f32, tag=f"pqk{j}{c}",
                                 name=f"pqk{j}{c}", bufs=1)
                    for j in range(2) for c in range(NSG)
                ]
                for ep in range(NEP):
                    for j in range(2):
                        for c in range(NSG):
                            nc.tensor.matmul(
                                ps_qk[j * NSG + c],
                                w8_js[j][:, 2 * ep:2 * ep + 2, :],
                                x8_sb[:, 2 * ep:2 * ep + 2, c * SG:(c + 1) * SG],
                                start=(ep == 0),
                                stop=(ep == NEP - 1),
                                perf_mode=DR,
                            )
                for j in range(2):
                    for c in range(NSG):
                        nc.any.tensor_copy(
                            qkT_sb[:, j, c * SG:(c + 1) * SG],
                            ps_qk[j * NSG + c],
                        )

            # -------- flat attention pipeline --------
            with (
                tc.tile_pool(name="att_ps", bufs=3, space="PSUM") as att_ps,
                tc.tile_pool(name="acc_ps", bufs=1, space="PSUM") as acc_ps,
                tc.tile_pool(name="work", bufs=2) as work_pool,
            ):
                # per-sg state created lazily
                state = {}

                def sg_state(sg):
                    if sg not in state:
                        state[sg] = dict(
                            out=acc_ps.tile([P, SG], f32, tag="out",
                                            name=f"out{sg}"),
                            den=acc_ps.tile([P, SG], f32, tag="den",
                                            name=f"den{sg}"),
                            expTs={}, attds={},
                        )
                    return state[sg]

                def emit_pairs(sg):
                    st = sg_state(sg)
                    s_sl = slice(sg * SG, (sg + 1) * SG)
                    keep_sg = keeps[sg]
                    for i in range(NPAIR):
                        ps = att_ps.tile([P, 2, SG], f32, tag="att",
                                         name=f"att{sg}_{i}")
                        for h_ in range(2):
                            t = 2 * i + h_
                            nc.tensor.matmul(
                                ps[:, h_, :],
                                qkT_sb[:, 1, t * P:(t + 1) * P],
                                qkT_sb[:, 0, s_sl],
                                start=True,
                                stop=True,
                            )
                        expT = work_pool.tile([P, 2, SG], f16, tag="exp",
                                              name=f"exp{sg}_{i}", bufs=17)
                        nc.scalar.activation(expT, ps, Exp, scale=SCALE)
                        attd = work_pool.tile([P, 2, SG], f16, tag="attd",
                                              name=f"attd{sg}_{i}", bufs=17)
                        nc.vector.tensor_mul(
                            out=attd, in0=expT,
                            in1=keep_sg[:, 2 * i:2 * i + 2, :],
                        )
                        st['expTs'][i] = expT
                        st['attds'][i] = attd

                def emit_post(sg):
                    st = sg_state(sg)
                    s_sl = slice(sg * SG, (sg + 1) * SG)
                    psum_out, psum_den = st['out'], st['den']
                    nc.vector.memset(psum_den, 0.0)
                    # den waves: 4 packed M=1 matmuls per wave, operands all
                    # long-done -> issue back-to-back and overlap
                    for w in range(NPAIR // 2):
                        e0 = st['expTs'].pop(2 * w)
                        e1 = st['expTs'].pop(2 * w + 1)
                        for j in range(4):
                            src = (e0 if j < 2 else e1)[:, j % 2, :]
                            nc.tensor.matmul(
                                psum_den[32 * j:32 * j + 1, :],
                                ones_t,
                                src,
                                start=(w == 0),
                                stop=(w == NPAIR // 2 - 1),
                                tile_position=(0, 32 * j),
                            )
                    for i in range(NPAIR):
                        attd = st['attds'].pop(i)
                        for h_ in range(2):
                            t = 2 * i + h_
                            nc.tensor.matmul(
                                psum_out,
                                v_sb[:, t, :],
                                attd[:, h_, :],
                                start=(t == 0),
                                stop=(t == NT - 1),
                            )
                    den_all = work_pool.tile([P, SG], f16, tag="den_all")
                    nc.vector.tensor_copy(den_all, psum_den)
                    nc.tensor.matmul(
                        psum_den, sel128, den_all, start=True, stop=True)
                    recip_sb = work_pool.tile([P, SG], f32, tag="recip")
                    nc.vector.reciprocal_approx_fast(
                        out=recip_sb, in_=psum_den)
                    out_sb = work_pool.tile([P, SG], f16, tag="out_sb")
                    nc.vector.tensor_mul(
                        out=out_sb, in0=psum_out, in1=recip_sb)
                    nc.sync.dma_start(outT_d[:, s_sl], out_sb)
                    del state[sg]

                def emit_v_proj():
                    # v projection in the out/den banks (not needed yet):
                    # two passes of two e-major chains, e in DMA-arrival order
                    order = [0, 1, 4, 5, 2, 3, 6, 7]
                    st0 = sg_state(0)
                    for half in range(2):
                        chains = [st0['out'], st0['den']]
                        for ei, e in enumerate(order):
                            for c2 in range(2):
                                c = 2 * half + c2
                                nc.tensor.matmul(
                                    chains[c2],
                                    wv_sb[:, e, :],
                                    xT_sb[:, e, c * SG:(c + 1) * SG],
                                    start=(ei == 0),
                                    stop=(ei == NE - 1),
                                )
                        for c2 in range(2):
                            c = 2 * half + c2
                            nc.any.tensor_copy(
                                vT_sb[:, c * SG:(c + 1) * SG], chains[c2])
                    # transposes: scratch = the same banks viewed as fp16
                    for g in range(NSG):
                        bank = st0['out'] if g % 2 else st0['den']
                        trv = bank[:].bitcast(f16)
                        for j in range(4):
                            nc.tensor.transpose(
                                trv[:, j * P:(j + 1) * P],
                                vT_sb[:, (4 * g + j) * P:(4 * g + j + 1) * P],
                                identity16,
                            )
                        nc.any.tensor_copy(
                            v_sb[:, 4 * g:4 * g + 4, :], trv[:, 0:4 * P])

                # pipeline: pairs(0), v, pairs(1), post(0), pairs(2),
                # post(1), pairs(3), post(2), post(3)
                emit_pairs(0)
                emit_v_proj()
                emit_pairs(1)
                emit_post(0)
                emit_pairs(2)
                emit_post(1)
                emit_pairs(3)
                emit_post(2)
                emit_post(3)

    nc.compile()
    _program_cache[key] = nc
    return nc


def kernel(x, wq, wk, wv, drop_u):
    from concourse import bass_utils

    x = np.asarray(x)
    wq = np.asarray(wq)
    wk = np.asarray(wk)
    wv = np.asarray(wv)
    drop_u = np.asarray(drop_u)

    nc = _build_program()
    in_maps = build_in_maps(x, wq, wk, wv, drop_u)
    last_err = None
    for _attempt in range(3):
        try:
            res = bass_utils.run_bass_kernel_spmd(
                nc, in_maps, core_ids=list(range(B)), trace=False
            )
            return np.stack(
                [np.asarray(res.results[b]["outT"]).T.astype(np.float32)
                 for b in range(B)],
                axis=0,
            )
        except Exception as e:  # transient device errors — retry
            last_err = e
            import time as _time

            _time.sleep(2.0)
    raise last_err


def _arrange_pe(a, ne):
    """[E, N] -> [128, ne, N] with e-chunk rows contiguous per partition."""
    E_, N_ = a.shape
    return np.ascontiguousarray(a.reshape(ne, P, N_).transpose(1, 0, 2))


def build_in_maps(x, wq, wk, wv, drop_u):
    f8 = ml_dtypes.float8_e4m3
    NE = E // P
    NT = S // P
    wq8 = _arrange_pe((np.asarray(wq) * W_SCALE).astype(f8), NE)
    wk8 = _arrange_pe((np.asarray(wk) * W_SCALE).astype(f8), NE)
    wv16 = _arrange_pe(np.asarray(wv).astype(np.float16), NE)
    in_maps = []
    for b in range(B):
        xTb = np.ascontiguousarray(x[b].T)
        x8 = _arrange_pe(xTb.astype(f8), NE)
        xT = _arrange_pe(xTb.astype(np.float16), NE)
        keep = _arrange_pe(
            (drop_u[b].T >= np.float32(DROP_P)).astype(np.float16), NT)
        in_maps.append(
            {"x8": x8, "xT": xT, "keep": keep,
             "wq8": wq8, "wk8": wk8, "wv": wv16}
        )
    return in_maps


# revision 27
# speedup vs baseline: 1.2624x; 1.1697x over previous
"""Single-head attention (B=8, S=2048, E=1024, H=128) with softmax + deterministic
dropout, data-parallel over batch across 8 NeuronCores (one batch element per core).

Layout ("transposed attention"), engineered so the ACT engine (which does the
4M-element exp() per core, the hard throughput floor at ~27us) never bubbles:

  - host ships x with the projection weights CONCATENATED per e-chunk row:
    xw8 fp8e4m3 [128, NE, S+2H] = x8 rows ++ wq8 ++ wk8 (weights pre-scaled by
    32 for fp8 subnormal avoidance, folded into the exp scale), and
    xwv fp16 [128, NE, S+H] = xT rows ++ wv.  One contiguous stream per dtype:
    no separate (slow) small weight DMAs, and the first q,k matmul can start
    as soon as the first e-pair lands.
  - q,k projections are fp8 DoubleRow matmuls (K=256/instruction, 2x PE rate);
    v projection is fp16.
  - dropout mask shipped {0,1} fp16 (keeps the wide DVE multiply in 2x 16-bit
    mode; the 1/(1-p) factor is folded into the denominator select matmul).
    The mask/xwv/output queue (Sync) is GATED behind the first q,k eviction
    via a dummy copy into the first mask tile, so it cannot steal HBM
    bandwidth from the projection-critical fp8 stream at startup.
  - attention is one flat stream of t-pairs: per pair one 2-bank PSUM tile
    takes two QK matmuls, one wide ACT exp -> expT fp16, one wide DVE mask
    multiply.  Each s-group's post-work (denominator waves, AV matmuls,
    normalize) is INTERLEAVED into the NEXT s-group's pair emission, so the
    PE always has exp-feeding pair matmuls in flight and ACT stays saturated
    across s-group boundaries.  The v projection is interleaved into
    s-group 0's pairs the same way, using the out/den PSUM banks (idle until
    the first AV matmul) for its accumulators and transpose scratch.
  - denominator: M=1 ones-matmuls per expT chunk in waves of 4 packed into
    distinct PE column groups (tile_position), emitted when operands are
    long-ready so they issue back-to-back and overlap in the PE array.
  - normalize: den partials in PSUM rows {0,32,64,96}; one select matmul sums
    and broadcasts 0.9*den[s]; reciprocal_approx_fast + one DVE multiply
    rescale out^T during eviction.  Output stays [H, S] fp16; the host
    transposes back to [S, H] fp32.

PSUM (8 banks): 3 x [128,2,512] QK tiles (6) + out (1) + den (1), with the v
projection and its transpose scratch time-sharing out/den before the AVs.
"""

import sys

for _p in ("/opt/trn_rl_repo",):
    if _p not in sys.path:
        sys.path.append(_p)

import numpy as np
import ml_dtypes

B, S, E, H = 8, 2048, 1024, 128
DROP_P = 0.1
P = 128
W_SCALE = 32.0  # host premultiplies wq/wk by this before fp8 cast

_program_cache = {}


def _build_program(S=S, E=E):
    key = (S, E)
    if key in _program_cache:
        return _program_cache[key]
    NT = S // P       # t-chunks (16)
    NE = E // P       # e-chunks (8)
    NEP = NE // 2     # e-pairs for DoubleRow (4)
    SG = 512          # s-group width (one fp32 PSUM bank)
    NSG = S // SG     # 4
    NPAIR = NT // 2   # t-pairs per s-group (8)
    W8 = S + 2 * H    # xw8 row length
    WV = S + H        # xwv row length

    import concourse.bass as bass  # noqa: F401
    import concourse.mybir as mybir
    import concourse.tile as tile
    from concourse import bacc
    from concourse.masks import make_identity

    f32 = mybir.dt.float32
    f16 = mybir.dt.float16
    f8 = mybir.dt.float8e4
    Exp = mybir.ActivationFunctionType.Exp
    DR = mybir.MatmulPerfMode.DoubleRow
    SCALE = float(E) ** -0.5 / (W_SCALE * W_SCALE)

    nc = bacc.Bacc("TRN2", target_bir_lowering=False, debug=False)
    xw8_d = nc.dram_tensor("xw8", [P, NE, W8], f8, kind="ExternalInput").ap()
    xwv_d = nc.dram_tensor("xwv", [P, NE, WV], f16, kind="ExternalInput").ap()
    keep_d = nc.dram_tensor("keep", [P, NT, S], f16, kind="ExternalInput").ap()
    outT_d = nc.dram_tensor("outT", [H, S], f16, kind="ExternalOutput").ap()

    with tile.TileContext(nc) as tc:
        with (
            tc.tile_pool(name="consts", bufs=1) as consts,
            tc.tile_pool(name="xw", bufs=1) as xw_pool,
            tc.tile_pool(name="keep_pool", bufs=2) as keep_pool,
        ):
            identity16 = consts.tile([P, P], f16)
            make_identity(nc, identity16)
            ones_t = consts.tile([P, 1], f16)
            nc.vector.memset(ones_t, 1.0)
            sel128 = consts.tile([P, P], f16)
            nc.vector.memset(sel128, 0.0)
            for j in range(4):
                nc.vector.memset(sel128[32 * j:32 * j + 1, :], 1.0 - DROP_P)

            xw8_sb = xw_pool.tile([P, NE, W8], f8, tag="xw8", name="xw8")
            xwv_sb = xw_pool.tile([P, NE, WV], f16, tag="xwv", name="xwv")
            qkT_sb = xw_pool.tile([P, 2, S], f16, tag="qkT", name="qkT")
            vT_sb = xw_pool.tile([P, S], f16, tag="vT", name="vT")
            v_sb = xw_pool.tile([P, NT, H], f16, tag="v", name="v")

            # qAct (scalar) queue: the projection-critical fp8 stream, then
            # the tail of xwv
            for ep in range(NEP):
                nc.scalar.dma_start(
                    xw8_sb[:, 2 * ep:2 * ep + 2, :], xw8_d[:, 2 * ep:2 * ep + 2, :]
                )
            for ep in (2, 3):
                nc.scalar.dma_start(
                    xwv_sb[:, 2 * ep:2 * ep + 2, :], xwv_d[:, 2 * ep:2 * ep + 2, :]
                )

            keeps = {}

            def fetch_keep(sg, engine, half=None):
                if sg not in keeps:
                    keeps[sg] = keep_pool.tile([P, NT, SG], f16, tag="keep",
                                               name=f"keep{sg}")
                t_ = keeps[sg]
                sl = slice(sg * SG, (sg + 1) * SG)
                if half is None:
                    engine.dma_start(t_, keep_d[:, :, sl])
                else:
                    rows = slice(0, NT // 2) if half == 0 else slice(NT // 2, NT)
                    engine.dma_start(t_[:, rows, :], keep_d[:, rows, sl])

            # Gate the Sync queue behind the first q,k eviction: this dummy
            # copy writes a corner of the first mask tile, so the mask DMA
            # (and everything queued after it) cannot start before the fp8
            # projection stream has fully landed.
            keeps[0] = keep_pool.tile([P, NT, SG], f16, tag="keep", name="keep0")
            nc.vector.tensor_copy(keeps[0][0:1, 0:1, 0:1], qkT_sb[0:1, 0:1, 0:1])
            fetch_keep(0, nc.sync, half=0)
            for ep in (0, 1):
                nc.sync.dma_start(
                    xwv_sb[:, 2 * ep:2 * ep + 2, :], xwv_d[:, 2 * ep:2 * ep + 2, :]
                )
            fetch_keep(0, nc.sync, half=1)
            fetch_keep(1, nc.sync)
            fetch_keep(2, nc.sync)
            fetch_keep(3, nc.sync)

            # -------- q,k projections: fp8 DoubleRow, e-pair-major --------
            with tc.tile_pool(name="proj_ps", bufs=8, space="PSUM") as proj_ps:
                ps_qk = [
                    proj_ps.tile([P, SG], f32, tag=f"pqk{j}{c}",
                                 name=f"pqk{j}{c}", bufs=1)
                    for j in range(2) for c in range(NSG)
                ]
                for ep in range(NEP):
                    for j in range(2):
                        for c in range(NSG):
                            nc.tensor.matmul(
                                ps_qk[j * NSG + c],
                                xw8_sb[:, 2 * ep:2 * ep + 2,
                                       S + j * H:S + (j + 1) * H],
                                xw8_sb[:, 2 * ep:2 * ep + 2, c * SG:(c + 1) * SG],
                                start=(ep == 0),
                                stop=(ep == NEP - 1),
                                perf_mode=DR,
                            )
                for j in range(2):
                    for c in range(NSG):
                        nc.any.tensor_copy(
                            qkT_sb[:, j, c * SG:(c + 1) * SG],
                            ps_qk[j * NSG + c],
                        )

            # -------- flat attention pipeline --------
            with (
                tc.tile_pool(name="att_ps", bufs=3, space="PSUM") as att_ps,
                tc.tile_pool(name="acc_ps", bufs=1, space="PSUM") as acc_ps,
                tc.tile_pool(name="work", bufs=2) as work_pool,
            ):
                state = {}

                def sg_state(sg):
                    if sg not in state:
                        state[sg] = dict(
                            out=acc_ps.tile([P, SG], f32, tag="out",
                                            name=f"out{sg}"),
                            den=acc_ps.tile([P, SG], f32, tag="den",
                                            name=f"den{sg}"),
                            expTs={}, attds={},
                        )
                    return state[sg]

                def emit_pair(sg, i):
                    st = sg_state(sg)
                    s_sl = slice(sg * SG, (sg + 1) * SG)
                    ps = att_ps.tile([P, 2, SG], f32, tag="att",
                                     name=f"att{sg}_{i}")
                    for h_ in range(2):
                        t = 2 * i + h_
                        nc.tensor.matmul(
                            ps[:, h_, :],
                            qkT_sb[:, 1, t * P:(t + 1) * P],
                            qkT_sb[:, 0, s_sl],
                            start=True,
                            stop=True,
                        )
                    expT = work_pool.tile([P, 2, SG], f16, tag="exp",
                                          name=f"exp{sg}_{i}", bufs=17)
                    nc.scalar.activation(expT, ps, Exp, scale=SCALE)
                    attd = work_pool.tile([P, 2, SG], f16, tag="attd",
                                          name=f"attd{sg}_{i}", bufs=17)
                    nc.vector.tensor_mul(
                        out=attd, in0=expT,
                        in1=keeps[sg][:, 2 * i:2 * i + 2, :],
                    )
                    st['expTs'][i] = expT
                    st['attds'][i] = attd

                def emit_av(st, i):
                    attd = st['attds'].pop(i)
                    for h_ in range(2):
                        t = 2 * i + h_
                        nc.tensor.matmul(
                            st['out'],
                            v_sb[:, t, :],
                            attd[:, h_, :],
                            start=(t == 0),
                            stop=(t == NT - 1),
                        )

                def emit_wave(st, w):
                    e0 = st['expTs'].pop(2 * w)
                    e1 = st['expTs'].pop(2 * w + 1)
                    for j in range(4):
                        src = (e0 if j < 2 else e1)[:, j % 2, :]
                        nc.tensor.matmul(
                            st['den'][32 * j:32 * j + 1, :],
                            ones_t,
                            src,
                            start=(w == 0),
                            stop=(w == NPAIR // 2 - 1),
                            tile_position=(0, 32 * j),
                        )

                def emit_norm(sg):
                    st = state[sg]
                    s_sl = slice(sg * SG, (sg + 1) * SG)
                    den_all = work_pool.tile([P, SG], f16, tag="den_all")
                    nc.vector.tensor_copy(den_all, st['den'])
                    nc.tensor.matmul(
                        st['den'], sel128, den_all, start=True, stop=True)
                    recip_sb = work_pool.tile([P, SG], f32, tag="recip")
                    nc.vector.reciprocal_approx_fast(
                        out=recip_sb, in_=st['den'])
                    out_sb = work_pool.tile([P, SG], f16, tag="out_sb")
                    nc.vector.tensor_mul(
                        out=out_sb, in0=st['out'], in1=recip_sb)
                    nc.sync.dma_start(outT_d[:, s_sl], out_sb)
                    del state[sg]

                # v projection pieces, interleaved into s-group 0's pairs.
                # e consumed in DMA-arrival order; accumulators live in the
                # out/den banks (2 chains per pass).
                V_ORDER = [4, 5, 6, 7, 0, 1, 2, 3]

                def emit_v_part(i):
                    st0 = sg_state(0)
                    chains = [st0['out'], st0['den']]
                    half, step = divmod(i, 4)
                    for e2 in range(2):
                        ei = 2 * step + e2
                        e = V_ORDER[ei]
                        for c2 in range(2):
                            c = 2 * half + c2
                            nc.tensor.matmul(
                                chains[c2],
                                xwv_sb[:, e, S:S + H],
                                xwv_sb[:, e, c * SG:(c + 1) * SG],
                                start=(ei == 0),
                                stop=(ei == NE - 1),
                            )
                    if step == 3:
                        for c2 in range(2):
                            c = 2 * half + c2
                            nc.any.tensor_copy(
                                vT_sb[:, c * SG:(c + 1) * SG], chains[c2])

                def emit_v_transposes():
                    st0 = sg_state(0)
                    for g in range(NSG):
                        bank = st0['out'] if g % 2 else st0['den']
                        trv = bank[:].bitcast(f16)
                        for j in range(4):
                            nc.tensor.transpose(
                                trv[:, j * P:(j + 1) * P],
                                vT_sb[:, (4 * g + j) * P:(4 * g + j + 1) * P],
                                identity16,
                            )
                        nc.any.tensor_copy(
                            v_sb[:, 4 * g:4 * g + 4, :], trv[:, 0:4 * P])

                # blocks: pairs(n) with post(n-1) interleaved
                for n in range(NSG):
                    prev = state.get(n - 1)
                    for i in range(NPAIR):
                        emit_pair(n, i)
                        if n == 0:
                            emit_v_part(i)
                        else:
                            if i == 0:
                                nc.vector.memset(prev['den'], 0.0)
                            emit_av(prev, i)
                            if i % 2 == 1:
                                emit_wave(prev, (i - 1) // 2)
                    if n == 0:
                        emit_v_transposes()
                    else:
                        emit_norm(n - 1)

                # tail: finish s-group 3 (its out/den banks are free only
                # after s-group 2's normalize above)
                st3 = sg_state(NSG - 1)
                nc.vector.memset(st3['den'], 0.0)
                for i in range(NPAIR):
                    emit_av(st3, i)
                    if i % 2 == 1:
                        emit_wave(st3, (i - 1) // 2)
                emit_norm(NSG - 1)

    nc.compile()
    _program_cache[key] = nc
    return nc


def kernel(x, wq, wk, wv, drop_u):
    from concourse import bass_utils

    x = np.asarray(x)
    wq = np.asarray(wq)
    wk = np.asarray(wk)
    wv = np.asarray(wv)
    drop_u = np.asarray(drop_u)

    nc = _build_program()
    in_maps = build_in_maps(x, wq, wk, wv, drop_u)
    last_err = None
    for _attempt in range(3):
        try:
            res = bass_utils.run_bass_kernel_spmd(
                nc, in_maps, core_ids=list(range(B)), trace=False
            )
            return np.stack(
                [np.asarray(res.results[b]["outT"]).T.astype(np.float32)
                 for b in range(B)],
                axis=0,
            )
        except Exception as e:  # transient device errors — retry
            last_err = e
            import time as _time

            _time.sleep(2.0)
    raise last_err


def _arrange_pe(a, ne):
    """[E, N] -> [128, ne, N] with e-chunk rows contiguous per partition."""
    E_, N_ = a.shape
    return np.ascontiguousarray(a.reshape(ne, P, N_).transpose(1, 0, 2))


def build_in_maps(x, wq, wk, wv, drop_u):
    f8 = ml_dtypes.float8_e4m3
    NE = E // P
    NT = S // P
    wq8 = _arrange_pe((np.asarray(wq) * W_SCALE).astype(f8), NE)
    wk8 = _arrange_pe((np.asarray(wk) * W_SCALE).astype(f8), NE)
    wv16 = _arrange_pe(np.asarray(wv).astype(np.float16), NE)
    in_maps = []
    for b in range(B):
        xTb = np.ascontiguousarray(x[b].T)
        x8 = _arrange_pe(xTb.astype(f8), NE)
        xT = _arrange_pe(xTb.astype(np.float16), NE)
        xw8 = np.concatenate([x8, wq8, wk8], axis=2)
        xwv = np.concatenate([xT, wv16], axis=2)
        keep = _arrange_pe(
            (drop_u[b].T >= np.float32(DROP_P)).astype(np.float16), NT)
        in_maps.append({"xw8": xw8, "xwv": xwv, "keep": keep})
    return in_maps
